# revision 9
# baseline (speedup 1.0000x reference)
"""MiniMaxText01 linear attention on 8 trn2 NeuronCores — mixed fp16/fp8.

Sharding: core c -> batch b = c//4, head-quad q = c%4 (4 heads per core,
single merged pass). Row-parallel out_proj emits one fp16 partial
[S, HID] per core; the host sums 4 partials per batch.

Precision (chosen from measured per-stage error/speed tradeoffs;
fp8e4m3 DoubleRow = 2x tensor throughput on TRN2):
  - q/k projections + out_proj: fp16 (error-critical paths)
  - v/gate projections: fp8 DoubleRow (errors attenuated downstream)
  - attention intra-chunk o and kv-update: fp8 DoubleRow over the
    256-token contraction; scores and kv*q stay fp16.
"""

import numpy as np
import ml_dtypes

B, S, HID = 2, 4096, 2048
H, D, C = 16, 128, 256
BLK = 512              # token block (2 chunks)
NBLK = S // BLK        # 8
KO = HID // 128        # 16 fp16 contraction subtiles
KO2 = HID // 256       # 8 fp8 DoubleRow slab-pairs
NCORES = 8
P = 128

E4NP = ml_dtypes.float8_e4m3   # HW float8e4 semantics (max 240)

_PROG = None


def _build_program():
    import concourse.bacc as bacc
    import concourse.mybir as mybir
    import concourse.tile as tile

    F32 = mybir.dt.float32
    F16 = mybir.dt.float16
    F8 = mybir.dt.float8e4
    AF = mybir.ActivationFunctionType
    MUL = mybir.AluOpType.mult
    ADD = mybir.AluOpType.add
    DR = mybir.MatmulPerfMode.DoubleRow

    nc = bacc.Bacc("TRN2", target_bir_lowering=False, debug=False,
                   num_devices=NCORES)

    x16_d = nc.dram_tensor("x16", [NBLK, P, KO, BLK], F16,
                           kind="ExternalInput")
    x8_d = nc.dram_tensor("x8", [NBLK, P, KO2, 2, BLK], F8,
                          kind="ExternalInput")
    wq_d = nc.dram_tensor("wq", [P, KO, 512], F16, kind="ExternalInput")
    wk_d = nc.dram_tensor("wk", [P, KO, 512], F16, kind="ExternalInput")
    wv_d = nc.dram_tensor("wv", [P, KO2, 2, 512], F8, kind="ExternalInput")
    wg_d = nc.dram_tensor("wg", [P, KO2, 2, 512], F8, kind="ExternalInput")
    wo_d = nc.dram_tensor("wo", [P, 4, HID], F16, kind="ExternalInput")
    ddT_d = nc.dram_tensor("ddT", [4, 2, P, C], F16, kind="ExternalInput")
    qdec_d = nc.dram_tensor("qdec", [4, P, C], F16, kind="ExternalInput")
    kdecT_d = nc.dram_tensor("kdecT", [P, 8], F32, kind="ExternalInput")
    bdec_d = nc.dram_tensor("bdec", [P, 4], F32, kind="ExternalInput")
    idn_d = nc.dram_tensor("idn", [P, P], F16, kind="ExternalInput")
    outd = nc.dram_tensor("out", [S, HID], F16, kind="ExternalOutput")

    with tile.TileContext(nc) as tc:
        with tc.tile_pool(name="const", bufs=1) as cpool, \
             tc.tile_pool(name="wpool", bufs=1) as wpool, \
             tc.tile_pool(name="xpool", bufs=2) as xpool, \
             tc.tile_pool(name="qkpool", bufs=2) as qkpool, \
             tc.tile_pool(name="apool", bufs=2) as apool, \
             tc.tile_pool(name="kvpool", bufs=1) as kvpool, \
             tc.tile_pool(name="opool", bufs=3) as opool, \
             tc.tile_pool(name="pproj", bufs=2, space="PSUM") as pproj, \
             tc.tile_pool(name="pattn", bufs=3, space="PSUM") as pattn, \
             tc.tile_pool(name="ptr", bufs=1, space="PSUM") as ptr, \
             tc.tile_pool(name="pout", bufs=2, space="PSUM") as pout:

            def load_x(blk):
                x16 = xpool.tile([P, KO, BLK], F16, tag="x16", name="x16")
                for hf in range(2):
                    nc.sync.dma_start(x16[:, hf * 8:(hf + 1) * 8, :],
                                      x16_d.ap()[blk, :, hf * 8:(hf + 1) * 8])
                x8 = xpool.tile([P, KO2, 2, BLK], F8, tag="x8", name="x8")
                nc.sync.dma_start(x8[:], x8_d.ap()[blk])
                return x16, x8

            cur_x = load_x(0)

            wq_s = wpool.tile([P, KO, 512], F16, tag="wq")
            wk_s = wpool.tile([P, KO, 512], F16, tag="wk")
            for hf in range(2):
                nc.sync.dma_start(wq_s[:, hf * 8:(hf + 1) * 8, :],
                                  wq_d.ap()[:, hf * 8:(hf + 1) * 8])
            for hf in range(2):
                nc.sync.dma_start(wk_s[:, hf * 8:(hf + 1) * 8, :],
                                  wk_d.ap()[:, hf * 8:(hf + 1) * 8])
            wv_s = wpool.tile([P, KO2, 2, 512], F8, tag="wv")
            nc.sync.dma_start(wv_s[:], wv_d.ap())
            wg_s = wpool.tile([P, KO2, 2, 512], F8, tag="wg")
            nc.sync.dma_start(wg_s[:], wg_d.ap())
            wo_s = wpool.tile([P, 4, HID], F16, tag="wo")
            for hf in range(2):
                nc.sync.dma_start(wo_s[:, hf * 2:(hf + 1) * 2, :],
                                  wo_d.ap()[:, hf * 2:(hf + 1) * 2])

            dd_sb = cpool.tile([P, 4, 2, C], F16)
            qd_sb = cpool.tile([P, 4, C], F16)
            for lh in range(4):
                for jt in range(2):
                    nc.sync.dma_start(dd_sb[:, lh, jt, :], ddT_d.ap()[lh, jt])
                nc.sync.dma_start(qd_sb[:, lh, :], qdec_d.ap()[lh])
            kdT = cpool.tile([P, 8], F32)
            nc.sync.dma_start(kdT[:], kdecT_d.ap())
            bd_sb = cpool.tile([P, 4], F32)
            nc.sync.dma_start(bd_sb[:], bdec_d.ap())
            ident = cpool.tile([P, P], F16)
            nc.sync.dma_start(ident[:], idn_d.ap())

            kv_sb = kvpool.tile([P, 4, P], F32, tag="kv")
            nc.vector.memset(kv_sb[:], 0.0)

            def proj_qk(wt, x16_t, dst):
                for m in range(4):
                    ps = pproj.tile([P, BLK], F32, tag="proj", name="ps")
                    for ko in range(KO):
                        nc.tensor.matmul(ps[:],
                                         wt[:, ko, m * P:(m + 1) * P],
                                         x16_t[:, ko, :],
                                         start=(ko == 0),
                                         stop=(ko == KO - 1))
                    nc.scalar.activation(dst[:, m, :], ps[:], AF.Silu)

            def proj_v(x8_t, dst):
                for m in range(4):
                    ps = pproj.tile([P, BLK], F32, tag="proj", name="ps")
                    for kk in range(KO2):
                        nc.tensor.matmul(ps[:],
                                         x8_t[:, kk, :, m * P:(m + 1) * P],
                                         wv_s[:, kk, :, :],
                                         start=(kk == 0),
                                         stop=(kk == KO2 - 1), perf_mode=DR)
                    nc.scalar.activation(dst[:, m, :], ps[:], AF.Silu,
                                         scale=1.0 / 64.0)

            def proj_g(x8_t, dst):
                for m in range(4):
                    ps = pproj.tile([P, BLK], F32, tag="proj", name="ps")
                    for kk in range(KO2):
                        nc.tensor.matmul(ps[:],
                                         wg_s[:, kk, :, m * P:(m + 1) * P],
                                         x8_t[:, kk, :, :],
                                         start=(kk == 0),
                                         stop=(kk == KO2 - 1), perf_mode=DR)
                    nc.scalar.activation(dst[:, m, :], ps[:], AF.Sigmoid,
                                         scale=1.0 / 64.0)

            def alloc_tiles():
                return {
                    "q": qkpool.tile([P, 4, BLK], F16, tag="qsb", name="qsb"),
                    "k": qkpool.tile([P, 4, BLK], F16, tag="ksb", name="ksb"),
                    "g": qkpool.tile([P, 4, BLK], F16, tag="gsb", name="gsb"),
                    "v": qkpool.tile([P, 4, BLK], F8, tag="vsb", name="vsb"),
                }

            def attn_chunk(blk, ch, t, go):
                qsb, ksb, gsb, vsb = t["q"], t["k"], t["g"], t["v"]
                co = ch * C
                first_chunk = (blk == 0 and ch == 0)
                for lh in range(4):
                    sm = apool.tile([P, 2, C], F8, tag="sm", name="sm")
                    for jt in range(2):
                        st = pattn.tile([P, C], F32, tag="sc", name="st")
                        nc.tensor.matmul(
                            st[:],
                            ksb[:, lh, co + jt * P:co + (jt + 1) * P],
                            qsb[:, lh, co:co + C], start=True, stop=True)
                        nc.vector.tensor_tensor(sm[:, jt, :], st[:],
                                                dd_sb[:, lh, jt, :], MUL)
                    ot = pattn.tile([P, C], F32, tag="sc", name="ot")
                    nc.tensor.matmul(ot[:],
                                     vsb[:, 2 * ch:2 * ch + 2,
                                         lh * P:(lh + 1) * P],
                                     sm[:, :, :], start=True,
                                     stop=first_chunk, perf_mode=DR)
                    if not first_chunk:
                        qdq = apool.tile([P, C], F16, tag="qdq", name="qdq")
                        nc.vector.tensor_tensor(qdq[:],
                                                qsb[:, lh, co:co + C],
                                                qd_sb[:, lh, :], MUL)
                        kvr = apool.tile([P, P], F16, tag="kvr", name="kvr")
                        nc.vector.tensor_copy(kvr[:], kv_sb[:, lh, :])
                        nc.tensor.matmul(ot[:], kvr[:], qdq[:],
                                         start=False, stop=True)
                    nc.vector.tensor_tensor(go[:, lh, co:co + C], ot[:],
                                            gsb[:, lh, co:co + C], MUL)
                    kn = apool.tile([P, 2, P], F8, tag="kn", name="kn")
                    for jt in range(2):
                        tp = ptr.tile([P, P], F16, tag="tr", name="tp")
                        nc.tensor.transpose(
                            tp[:],
                            ksb[:, lh, co + jt * P:co + (jt + 1) * P],
                            ident[:])
                        ci = lh * 2 + jt
                        nc.vector.tensor_scalar(kn[:, jt, :], tp[:],
                                                kdT[:, ci:ci + 1], None,
                                                MUL)
                    up = pattn.tile([P, C], F32, tag="sc", name="up")
                    nc.tensor.matmul(up[:, :P], kn[:, :, :],
                                     vsb[:, 2 * ch:2 * ch + 2,
                                         lh * P:(lh + 1) * P],
                                     start=True, stop=True, perf_mode=DR)
                    nc.vector.scalar_tensor_tensor(
                        kv_sb[:, lh, :], kv_sb[:, lh, :],
                        bd_sb[:, lh:lh + 1], up[:, :P], MUL, ADD)

            # software pipeline: next block's projections are emitted between
            # this block's attention chunks / out-proj so the PE never waits
            # on the vector engine at phase joins.
            cur_t = alloc_tiles()
            proj_qk(wq_s, cur_x[0], cur_t["q"])
            proj_qk(wk_s, cur_x[0], cur_t["k"])
            proj_v(cur_x[1], cur_t["v"])
            proj_g(cur_x[1], cur_t["g"])

            for blk in range(NBLK):
                t0 = blk * BLK
                last = blk + 1 >= NBLK
                nxt_x = None if last else load_x(blk + 1)
                nxt_t = None if last else alloc_tiles()

                go = qkpool.tile([P, 4, BLK], F16, tag="go", name="go")
                attn_chunk(blk, 0, cur_t, go)
                if not last:
                    proj_qk(wq_s, nxt_x[0], nxt_t["q"])
                attn_chunk(blk, 1, cur_t, go)
                if not last:
                    proj_qk(wk_s, nxt_x[0], nxt_t["k"])

                # ---- out projection: fp16 (row-parallel partial)
                for mt in range(4):
                    ob = opool.tile([P, 4, 512], F16, tag="ob", name="ob")
                    for nt in range(4):
                        po = pout.tile([P, 512], F32, tag="out", name="po")
                        for kh in range(4):
                            nc.tensor.matmul(
                                po[:], go[:, kh, mt * P:(mt + 1) * P],
                                wo_s[:, kh, nt * 512:(nt + 1) * 512],
                                start=(kh == 0), stop=(kh == 3))
                        nc.scalar.copy(ob[:, nt, :], po[:])
                    nc.sync.dma_start(
                        outd.ap()[t0 + mt * P:t0 + (mt + 1) * P, :], ob[:])

                if not last:
                    proj_v(nxt_x[1], nxt_t["v"])
                    proj_g(nxt_x[1], nxt_t["g"])
                    cur_x, cur_t = nxt_x, nxt_t

    nc.compile()
    return nc


def _get_program():
    global _PROG
    if _PROG is None:
        _PROG = _build_program()
    return _PROG


_XCACHE = {}


def _prep_x(x, b):
    if b not in _XCACHE:
        xT = np.ascontiguousarray(x[b].T.astype(np.float32))     # [HID, S]
        x16 = np.ascontiguousarray(
            xT.reshape(KO, P, NBLK, BLK).transpose(2, 1, 0, 3)
        ).astype(np.float16)
        x8 = np.ascontiguousarray(
            xT.reshape(KO2, 2, P, NBLK, BLK).transpose(3, 2, 0, 1, 4)
        ).astype(E4NP)
        _XCACHE[b] = {"x16": x16, "x8": x8}
    return _XCACHE[b]


def _prep_core_inputs(x, w_qkv, w_gate, w_out, slopes, core):
    b, q = core // 4, core % 4
    h0 = 4 * q
    s = np.asarray(slopes, dtype=np.float64).reshape(H)[h0:h0 + 4]

    d = dict(_prep_x(x, b))

    def wtile16(w2):
        # [HID, 512] -> [P, KO, 512]
        return np.ascontiguousarray(
            w2.astype(np.float32).reshape(KO, P, 512).transpose(1, 0, 2)
        ).astype(np.float16)

    def wtile8(w2):
        # [HID, 512] -> [P, KO2, 2, 512], scaled x64
        return np.ascontiguousarray(
            (w2.astype(np.float32) * 64.0)
            .reshape(KO2, 2, P, 512).transpose(2, 0, 1, 3)).astype(E4NP)

    cq = slice(h0 * D, h0 * D + 512)
    d["wq"] = wtile16(w_qkv[:, cq])
    d["wk"] = wtile16(w_qkv[:, 2048 + h0 * D:2048 + h0 * D + 512])
    d["wv"] = wtile8(w_qkv[:, 4096 + h0 * D:4096 + h0 * D + 512])
    d["wg"] = wtile8(w_gate[:, cq])
    # [512, HID] -> [P, 4, HID]
    d["wo"] = np.ascontiguousarray(
        w_out[cq, :].astype(np.float32).reshape(4, P, HID).transpose(1, 0, 2)
    ).astype(np.float16)

    pos = np.arange(C, dtype=np.float64)
    idx = pos[:, None] - pos[None, :]                     # i - j
    ddT = np.empty((4, 2, P, C), dtype=np.float16)
    qdec = np.empty((4, P, C), dtype=np.float16)
    kdecT = np.empty((P, 8), dtype=np.float32)
    bdec = np.empty((P, 4), dtype=np.float32)
    for lh in range(4):
        sh = s[lh]
        m = np.where(idx >= 0, np.exp(-sh * idx), 0.0)    # [i, j]
        ddT[lh] = m.T.reshape(2, P, C).astype(np.float16)
        qdec[lh] = np.broadcast_to(
            np.exp(-sh * (pos + 1.0)).astype(np.float16)[None, :], (P, C))
        for jt in range(2):
            jj = jt * P + np.arange(P, dtype=np.float64)
            kdecT[:, lh * 2 + jt] = np.exp(-sh * (C - 1.0 - jj))
        bdec[:, lh] = np.float32(np.exp(-sh * C))

    d.update(ddT=ddT, qdec=qdec, kdecT=kdecT,
             bdec=np.ascontiguousarray(bdec),
             idn=np.eye(P, dtype=np.float16))
    return d


def kernel(x, w_qkv, w_gate, w_out, slopes, _trace=False, _result_holder=None):
    from concourse.bass_utils import run_bass_kernel_spmd

    x = np.asarray(x, dtype=np.float32)
    w_qkv = np.asarray(w_qkv, dtype=np.float32)
    w_gate = np.asarray(w_gate, dtype=np.float32)
    w_out = np.asarray(w_out, dtype=np.float32)

    _XCACHE.clear()
    nc = _get_program()
    in_maps = [_prep_core_inputs(x, w_qkv, w_gate, w_out, slopes, c)
               for c in range(NCORES)]
    _XCACHE.clear()
    res = run_bass_kernel_spmd(nc, in_maps, core_ids=list(range(NCORES)),
                               trace=_trace)
    if _result_holder is not None:
        _result_holder.append(res)

    out = np.zeros((B, S, HID), dtype=np.float32)
    for c in range(NCORES):
        out[c // 4] += res.results[c]["out"].astype(np.float32)
    return out
